# revision 1
# baseline (speedup 1.0000x reference)
"""ConvMultiheadAttention Trainium2 kernel (8 NeuronCores).

Sharding: core c = (batch b = c//2) x (head-group hg = c%2, 8 heads each).
Per core:
  - q/k/v conv1d projections (K=3, same pad) for this core's 512 output
    channels, expressed as PSUM-accumulated bf16 matmuls over x laid out
    [c_in partitions, L free].
  - attention with TRANSPOSED scores sT[j, i] (j on partitions) so the
    key-padding mask folds into the Exp activation's per-partition bias,
    and the softmax denominator comes for free from a ones-augmented
    AV matmul (row 64 of the [65, i] psum = column sums).
  - partial out-conv contracting over this core's 512 attention-output
    channels; the host sums the two partials per batch.
Host folds: attention scale + q-bias into q-conv weights/bias; k-bias is
dropped (constant per softmax row -> cancels); v-bias and o-bias are
applied on the host after the gather (attention rows sum to 1).
"""

import os
import numpy as np
import ml_dtypes

BF16 = ml_dtypes.bfloat16

B, L, D = 4, 1024, 1024
NH, HD = 16, 64
KW = 3
NCORES = 8
HALF = D // 2  # channels per core half (8 heads)
SCALE = HD ** -0.5
MASK_BIAS = -30000.0

_CACHE = {}


def _build_nc():
    import concourse.bass as bass  # noqa: F401
    import concourse.tile as tile
    from concourse import bacc, mybir

    f32 = mybir.dt.float32
    bf16 = mybir.dt.bfloat16
    Act = mybir.ActivationFunctionType

    nc = bacc.Bacc(
        "TRN2",
        target_bir_lowering=False,
        debug=False,
        enable_asserts=False,
        num_devices=NCORES,
    )

    # ---- DRAM I/O ----
    xq_d = nc.dram_tensor("xq", [8, 128, L], bf16, kind="ExternalInput").ap()
    xk_d = nc.dram_tensor("xk", [8, 128, L], bf16, kind="ExternalInput").ap()
    xv_d = nc.dram_tensor("xv", [8, 128, L], bf16, kind="ExternalInput").ap()
    wq_d = nc.dram_tensor("wq", [4, 128, KW, 8, 128], bf16, kind="ExternalInput").ap()
    wk_d = nc.dram_tensor("wk", [4, 128, KW, 8, 128], bf16, kind="ExternalInput").ap()
    wv_d = nc.dram_tensor("wv", [4, 128, KW, 8, 128], bf16, kind="ExternalInput").ap()
    wo_d = nc.dram_tensor("wo", [8, 128, KW, 4, 128], bf16, kind="ExternalInput").ap()
    qb_d = nc.dram_tensor("qb", [128, 4], f32, kind="ExternalInput").ap()
    jb_d = nc.dram_tensor("jb", [128, 8], f32, kind="ExternalInput").ap()
    out_d = nc.dram_tensor("out", [8, 128, L], f32, kind="ExternalOutput").ap()

    from concourse.masks import make_identity

    with tile.TileContext(nc) as tc:
        with (
            tc.tile_pool(name="singles", bufs=1) as singles,
            tc.tile_pool(name="wpool", bufs=3) as wpool,
            tc.tile_pool(name="qk", bufs=2) as qkpool,
            tc.tile_pool(name="vpool", bufs=2) as vpool,
            tc.tile_pool(name="ppool", bufs=2) as ppool,
            tc.tile_pool(name="outp", bufs=4) as outp,
            tc.tile_pool(name="smalls", bufs=4) as smalls,
            tc.tile_pool(name="convp", bufs=2, space="PSUM") as convp,
            tc.tile_pool(name="scorep", bufs=2, space="PSUM") as scorep,
            tc.tile_pool(name="avp", bufs=2, space="PSUM") as avp,
        ):
            # ---- constants / resident tiles ----
            ident = singles.tile([128, 128], bf16, tag="ident")
            make_identity(nc, ident)
            qb_s = singles.tile([128, 4], f32, tag="qb")
            nc.sync.dma_start(qb_s, qb_d)
            jb_s = singles.tile([128, 8], f32, tag="jb")
            nc.sync.dma_start(jb_s, jb_d)

            xq_s = singles.tile([128, 8, L], bf16, tag="xq")
            xk_s = singles.tile([128, 8, L], bf16, tag="xk")
            xv_s = singles.tile([128, 8, L], bf16, tag="xv")
            for cc in range(8):
                nc.sync.dma_start(xv_s[:, cc, :], xv_d[cc])
            for cc in range(8):
                nc.sync.dma_start(xq_s[:, cc, :], xq_d[cc])
            for cc in range(8):
                nc.sync.dma_start(xk_s[:, cc, :], xk_d[cc])

            # vT[j_part, j_chunk, head, 0:64] = v[h*64+d, j]; col 64 = ones
            vT = singles.tile([128, 8, 8, 65], bf16, tag="vT")
            for h in range(8):
                nc.vector.memset(vT[:, :, h, 64:65], 1.0)
            o_x = singles.tile([128, 4, L], bf16, tag="ox")

            def conv_mms(ps, w_t, x_t, n_ci, lh):
                """Accumulate conv-as-matmul into psum ps[:, 0:512] for
                output columns [lh*512, lh*512+512)."""
                # center tap first: full-width start=True write covers the
                # whole bank, so the edge taps' partial-width writes are
                # pure accumulations (uniform has_written state).
                korder = [1, 0, 2]
                first = True
                for k in korder:
                    for cc in range(n_ci):
                        lo = lh * 512 + k - 1
                        lhsT = w_t[:, k, cc, :]
                        if lo < 0:
                            rhs = x_t[:, cc, 0:511]
                            outap = ps[:, 1:512]
                        elif lo + 512 > L:
                            rhs = x_t[:, cc, lo:L]
                            outap = ps[:, 0 : L - lo]
                        else:
                            rhs = x_t[:, cc, lo : lo + 512]
                            outap = ps[:, 0:512]
                        nc.tensor.matmul(
                            outap,
                            lhsT,
                            rhs,
                            start=first,
                            stop=(k == korder[-1] and cc == n_ci - 1),
                        )
                        first = False

            # ---- V conv + transpose into vT ----
            for occ in range(4):
                wv_t = wpool.tile([128, KW, 8, 128], bf16, tag="w")
                nc.sync.dma_start(wv_t, wv_d[occ])
                v_t = vpool.tile([128, L], bf16, tag="v")
                for lh in range(2):
                    ps = convp.tile([128, 512], f32, tag="cp")
                    conv_mms(ps, wv_t, xv_s, 8, lh)
                    nc.vector.tensor_copy(v_t[:, lh * 512 : (lh + 1) * 512], ps)
                for lb in range(8):
                    tp = convp.tile([128, 128], bf16, tag="cp")
                    nc.tensor.transpose(tp, v_t[:, lb * 128 : (lb + 1) * 128], ident)
                    nc.vector.tensor_copy(vT[:, lb, 2 * occ, 0:64], tp[:, 0:64])
                    nc.vector.tensor_copy(vT[:, lb, 2 * occ + 1, 0:64], tp[:, 64:128])

            # ---- per head-pair: software-pipelined q/k conv + attention ----
            # Per pair t: scores+exp(t) -> q/k conv(t+1) [PE work that hides
            # exp(t) on ACT] -> AV+normalize(t).
            def qk_conv_units(t):
                """Return (q_t, k_t, units): four closures each emitting one
                conv psum-group (~5us of dense PE work) for pair t."""
                q_t = qkpool.tile([128, L], bf16, tag="q", name=f"q{t}")
                k_t = qkpool.tile([128, L], bf16, tag="k", name=f"k{t}")
                state = {}

                def unit(which, lh):
                    if which == "q" and lh == 0:
                        state["wq"] = wpool.tile([128, KW, 8, 128], bf16, tag="w",
                                                 name="wqt")
                        nc.sync.dma_start(state["wq"], wq_d[t])
                    if which == "k" and lh == 0:
                        state["wk"] = wpool.tile([128, KW, 8, 128], bf16, tag="w",
                                                 name="wkt")
                        nc.sync.dma_start(state["wk"], wk_d[t])
                    ps = convp.tile([128, 512], f32, tag="cp")
                    if which == "q":
                        conv_mms(ps, state["wq"], xq_s, 8, lh)
                        nc.vector.tensor_scalar_add(
                            q_t[:, lh * 512 : (lh + 1) * 512], ps,
                            qb_s[:, t : t + 1])
                    else:
                        conv_mms(ps, state["wk"], xk_s, 8, lh)
                        nc.vector.tensor_copy(
                            k_t[:, lh * 512 : (lh + 1) * 512], ps)

                units = [lambda w=w, lh=lh: unit(w, lh)
                         for w in ("q", "k") for lh in range(2)]
                return q_t, k_t, units

            def qk_conv(t):
                q_t, k_t, units = qk_conv_units(t)
                for u in units:
                    u()
                return q_t, k_t

            _ablate = os.environ.get("KERNEL_ABLATE", "")
            q_t, k_t = qk_conv(0)
            if _ablate == "noattn":
                nc.vector.memset(o_x, 0.01)
                for t in range(1, 4):
                    q_t, k_t = qk_conv(t)
            for t in range(4 if _ablate != "noattn" else 0):
                # next pair's conv psum-groups, interleaved between score
                # jc-groups below so PE stays busy while ACT runs exp(t)
                if t < 3:
                    nq_t, nk_t, conv_units = qk_conv_units(t + 1)
                else:
                    conv_units = []
                # scores + exp for both heads; adjacent matmuls of the two
                # heads hit disjoint PE row groups (base 0 / 64) and overlap.
                p_pair = []
                for jc in range(8):
                    sps_pair = [scorep.tile([128, L], f32, tag="score",
                                            name=f"sps{hh2}")
                                for hh2 in range(2)]
                    if jc == 0:
                        p_pair = [ppool.tile([128, 8, L], bf16, tag="p",
                                             name=f"p{hh2}")
                                  for hh2 in range(2)]
                    for ih in range(2):
                        for hh in range(2):
                            base = hh * 64
                            nc.tensor.matmul(
                                sps_pair[hh][:, ih * 512 : (ih + 1) * 512],
                                k_t[base : base + 64, jc * 128 : (jc + 1) * 128],
                                q_t[base : base + 64, ih * 512 : (ih + 1) * 512],
                                start=True,
                                stop=True,
                            )
                    for hh in range(2):
                        nc.scalar.activation(
                            p_pair[hh][:, jc, :], sps_pair[hh], Act.Exp,
                            bias=jb_s[:, jc : jc + 1],
                        )
                    # one conv psum-group (~5us dense PE) after every other
                    # jc-group: fills the PE stall while ACT drains exp(t)
                    if jc % 2 == 1 and conv_units:
                        conv_units.pop(0)()
                # AV + normalize for both heads
                for hh in range(2):
                    h = 2 * t + hh
                    base = hh * 64
                    for ih in range(2):
                        avps = avp.tile([65, 512], f32, tag="av")
                        for jc in range(8):
                            nc.tensor.matmul(
                                avps,
                                vT[:, jc, h, :],
                                p_pair[hh][:, jc, ih * 512 : (ih + 1) * 512],
                                start=(jc == 0),
                                stop=(jc == 7),
                            )
                        r_t = smalls.tile([1, 512], f32, tag="r")
                        nc.vector.reciprocal(r_t, avps[64:65, :])
                        bc_t = smalls.tile([64, 512], f32, tag="bc")
                        nc.gpsimd.partition_broadcast(bc_t, r_t)
                        dst = o_x[base : base + 64, t, ih * 512 : (ih + 1) * 512]
                        if hh == 0:
                            nc.vector.tensor_mul(dst, avps[0:64, :], bc_t)
                        else:
                            tmp = smalls.tile([64, 512], bf16, tag="tmp")
                            nc.vector.tensor_mul(tmp, avps[0:64, :], bc_t)
                            nc.sync.dma_start(dst, tmp)
                if t < 3:
                    q_t, k_t = nq_t, nk_t

            # ---- out conv (partial over this core's 512 input channels) ----
            for occ in range(8 if _ablate != "nooconv" else 0):
                wo_t = wpool.tile([128, KW, 4, 128], bf16, tag="w")
                nc.sync.dma_start(wo_t, wo_d[occ])
                for lh in range(2):
                    ps = convp.tile([128, 512], f32, tag="cp")
                    conv_mms(ps, wo_t, o_x, 4, lh)
                    o_t = outp.tile([128, 512], f32, tag="osb")
                    nc.vector.tensor_copy(o_t, ps)
                    nc.sync.dma_start(out_d[occ, :, lh * 512 : (lh + 1) * 512], o_t)

    nc.compile()
    return nc


def _get_nc():
    if "nc" not in _CACHE:
        _CACHE["nc"] = _build_nc()
    return _CACHE["nc"]


def _prep_inputs(query, key, value, key_padding_mask, attn_mask,
                 q_w, q_b, k_w, k_b, v_w, v_b, o_w, o_b):
    """Build the 8 per-core input maps (host-side shard + layout)."""
    query = np.asarray(query, np.float32)
    key = np.asarray(key, np.float32)
    value = np.asarray(value, np.float32)
    kpm = np.asarray(key_padding_mask)
    attn_mask = np.asarray(attn_mask, np.float32)
    q_w = np.asarray(q_w, np.float32); q_b = np.asarray(q_b, np.float32)
    k_w = np.asarray(k_w, np.float32)
    v_w = np.asarray(v_w, np.float32)
    o_w = np.asarray(o_w, np.float32); o_b = np.asarray(o_b, np.float32)

    # attn_mask must be constant across query rows to fold into the key bias
    if not np.all(attn_mask == attn_mask[0:1, :]):
        raise NotImplementedError("attn_mask varying over query index unsupported")
    am_row = attn_mask[0]

    def conv_w_layout(w, occ, n_ci):
        # w: [C_out_part, C_in_part, KW] -> [occ, p(ci), k, cc, m(c_out)]
        co, ci, _ = w.shape
        arr = w.reshape(occ, 128, n_ci, 128, KW).transpose(0, 3, 4, 2, 1)
        return np.ascontiguousarray(arr).astype(BF16)

    wq_h, wk_h, wv_h, wo_h, qb_h = [], [], [], [], []
    for hg in range(2):
        sl = slice(hg * HALF, (hg + 1) * HALF)
        wq_h.append(conv_w_layout(q_w[sl] * SCALE, 4, 8))
        wk_h.append(conv_w_layout(k_w[sl], 4, 8))
        wv_h.append(conv_w_layout(v_w[sl], 4, 8))
        # out conv: contract over this half's input channels
        wo_h.append(conv_w_layout(o_w[:, sl, :], 8, 4))
        qb_h.append(np.ascontiguousarray(
            (q_b[sl] * SCALE).reshape(4, 128).T).astype(np.float32))

    xq_b, xk_b, xv_b, jb_b = [], [], [], []
    for b in range(B):
        xq_b.append(np.ascontiguousarray(query[b].T).reshape(8, 128, L).astype(BF16))
        xk_b.append(np.ascontiguousarray(key[b].T).reshape(8, 128, L).astype(BF16))
        xv_b.append(np.ascontiguousarray(value[b].T).reshape(8, 128, L).astype(BF16))
        jb = np.where(kpm[b], MASK_BIAS, 0.0).astype(np.float32) + am_row
        jb_b.append(np.ascontiguousarray(jb.reshape(8, 128).T).astype(np.float32))

    in_maps = []
    for c in range(NCORES):
        b, hg = c // 2, c % 2
        in_maps.append({
            "xq": xq_b[b], "xk": xk_b[b], "xv": xv_b[b],
            "wq": wq_h[hg], "wk": wk_h[hg], "wv": wv_h[hg], "wo": wo_h[hg],
            "qb": qb_h[hg], "jb": jb_b[b],
        })
    return in_maps, (o_w, np.asarray(v_b, np.float32), o_b)


def _postprocess(parts, extras):
    """parts: list of 8 arrays [8,128,L] f32 -> full output [B, L, D] f32."""
    o_w, v_b, o_b = extras
    # v-bias contribution through the out conv (attention rows sum to 1):
    # interior columns see all 3 taps, edge columns lose one.
    a_full = o_w.sum(axis=2) @ v_b            # [D]
    a_l0 = a_full - o_w[:, :, 0] @ v_b        # l = 0 loses tap k=0
    a_lL = a_full - o_w[:, :, 2] @ v_b        # l = L-1 loses tap k=2
    out = np.empty((B, L, D), np.float32)
    for b in range(B):
        tot = (parts[2 * b] + parts[2 * b + 1]).reshape(D, L)
        tot = tot + o_b[:, None] + a_full[:, None]
        tot[:, 0] += a_l0 - a_full
        tot[:, -1] += a_lL - a_full
        out[b] = tot.T
    return out


def _run(in_maps, trace=False, **kw):
    from concourse import bass_utils
    nc = _get_nc()
    try:
        res = bass_utils.run_bass_kernel_spmd(
            nc, in_maps, core_ids=list(range(NCORES)), trace=trace, **kw)
    except ModuleNotFoundError:
        # NTFF profiling hook unavailable (axon client without axon.trn);
        # rerun without trace.
        res = bass_utils.run_bass_kernel_spmd(
            nc, in_maps, core_ids=list(range(NCORES)), trace=False, **kw)
    return res


def kernel(**inputs) -> np.ndarray:
    in_maps, extras = _prep_inputs(**inputs)
    res = _run(in_maps, trace=bool(int(os.environ.get("KERNEL_TRACE", "0"))))
    parts = [res.results[c]["out"] for c in range(NCORES)]
    out = _postprocess(parts, extras)
    if res.exec_time_ns is not None:
        print(f"HW exec time: {res.exec_time_ns} ns")
    return out



# revision 4
# speedup vs baseline: 1.1406x; 1.1406x over previous
"""ConvMultiheadAttention Trainium2 kernel (8 NeuronCores).

Sharding: core c = (batch b = c//2) x (head-group hg = c%2, 8 heads each).
Per core:
  - q/k conv1d projections (K=3, same pad) for this core's 512 output
    channels, expressed as PSUM-accumulated bf16 matmuls over x laid out
    [c_in partitions, L free].
  - v conv computed TRANSPOSED (out = [j partitions, channel free]) by
    swapping matmul operands, so no PE transposes are needed to build the
    AV lhsT; column 64 of each head's 65-wide slot holds ones so the
    softmax denominator falls out of the AV matmul.
  - attention with TRANSPOSED scores sT[j, i] (j on partitions) so the
    key-padding mask folds into the Exp activation's per-partition bias.
  - partial out-conv contracting over this core's 512 attention-output
    channels; the host sums the two partials per batch.
DMA: x tensors stream on the SP HWDGE queue; all weights stream on the
Activation HWDGE queue (prefetched ahead of use) so the PE never waits
behind bulk input traffic.
Host folds: attention scale + q-bias into q-conv weights/bias; k-bias is
dropped (constant per softmax row -> cancels); v-bias and o-bias are
applied on the host after the gather (attention rows sum to 1).
"""

import os
import numpy as np
import ml_dtypes

BF16 = ml_dtypes.bfloat16

B, L, D = 4, 1024, 1024
NH, HD = 16, 64
KW = 3
NCORES = 8
HALF = D // 2  # channels per core half (8 heads)
SCALE = HD ** -0.5
MASK_BIAS = -30000.0

_CACHE = {}


def _build_nc():
    import concourse.bass as bass  # noqa: F401
    import concourse.tile as tile
    from concourse import bacc, mybir

    f32 = mybir.dt.float32
    bf16 = mybir.dt.bfloat16
    Act = mybir.ActivationFunctionType

    nc = bacc.Bacc(
        "TRN2",
        target_bir_lowering=False,
        debug=False,
        enable_asserts=False,
        num_devices=NCORES,
    )

    # ---- DRAM I/O ----
    xq_d = nc.dram_tensor("xq", [8, 128, L], bf16, kind="ExternalInput").ap()
    xk_d = nc.dram_tensor("xk", [8, 128, L], bf16, kind="ExternalInput").ap()
    xv_d = nc.dram_tensor("xv", [8, 128, L], bf16, kind="ExternalInput").ap()
    wq_d = nc.dram_tensor("wq", [4, 128, KW, 8, 128], bf16, kind="ExternalInput").ap()
    wk_d = nc.dram_tensor("wk", [4, 128, KW, 8, 128], bf16, kind="ExternalInput").ap()
    # v weights in transposed-conv layout: [ci_block, ci, k, c_local]
    wv_d = nc.dram_tensor("wv", [8, 128, KW, 512], bf16, kind="ExternalInput").ap()
    wo_d = nc.dram_tensor("wo", [8, 128, KW, 4, 128], bf16, kind="ExternalInput").ap()
    qb_d = nc.dram_tensor("qb", [128, 4], f32, kind="ExternalInput").ap()
    jb_d = nc.dram_tensor("jb", [128, 8], f32, kind="ExternalInput").ap()
    out_d = nc.dram_tensor("out", [8, 128, L], bf16, kind="ExternalOutput").ap()

    with tile.TileContext(nc) as tc:
        with (
            tc.tile_pool(name="singles", bufs=1) as singles,
            tc.tile_pool(name="wpool", bufs=5) as wpool,
            tc.tile_pool(name="qk", bufs=2) as qkpool,
            tc.tile_pool(name="ppool", bufs=2) as ppool,
            tc.tile_pool(name="outp", bufs=4) as outp,
            tc.tile_pool(name="smalls", bufs=4) as smalls,
            tc.tile_pool(name="convp", bufs=2, space="PSUM") as convp,
            tc.tile_pool(name="scorep", bufs=2, space="PSUM") as scorep,
            tc.tile_pool(name="avp", bufs=2, space="PSUM") as avp,
        ):
            # ---- resident tiles; x on SP queue, weights on ACT queue ----
            qb_s = singles.tile([128, 4], f32, tag="qb")
            nc.sync.dma_start(qb_s, qb_d)
            jb_s = singles.tile([128, 8], f32, tag="jb")
            nc.sync.dma_start(jb_s, jb_d)

            xq_s = singles.tile([128, 8, L], bf16, tag="xq")
            xk_s = singles.tile([128, 8, L], bf16, tag="xk")
            # xv zero-padded by one column on each side so the transposed
            # v-conv's shifted taps stay full-width (PE psum writes must
            # start at partition 0).
            xv_s = singles.tile([128, 8, L + 2], bf16, tag="xv")
            wv_s = singles.tile([128, 8, KW, 512], bf16, tag="wv")
            nc.vector.memset(xv_s[:, :, 0:1], 0.0)
            nc.vector.memset(xv_s[:, :, L + 1 : L + 2], 0.0)
            for cc in range(8):
                nc.sync.dma_start(xv_s[:, cc, 1 : L + 1], xv_d[cc])
            for cc in range(8):
                nc.scalar.dma_start(wv_s[:, cc, :, :], wv_d[cc])
            for cc in range(8):
                nc.sync.dma_start(xq_s[:, cc, :], xq_d[cc])
            for cc in range(8):
                nc.sync.dma_start(xk_s[:, cc, :], xk_d[cc])

            # vT[j_part, j_chunk, head, 0:64] = v[h*64+d, j]; col 64 = ones
            vT = singles.tile([128, 8, 8, 65], bf16, tag="vT")
            for h in range(8):
                nc.vector.memset(vT[:, :, h, 64:65], 1.0)
            o_x = singles.tile([128, 4, L], bf16, tag="ox")

            def conv_mms(ps, w_t, x_t, ccs, lh, start, stop):
                """Accumulate conv-as-matmul into psum ps[:, 0:512] for
                output columns [lh*512, lh*512+512) over blocks ccs."""
                # center tap first: full-width start=True write covers the
                # whole bank, so the edge taps' partial-width writes are
                # pure accumulations (uniform has_written state).
                items = [(k, cc) for k in (1, 0, 2) for cc in ccs]
                for idx, (k, cc) in enumerate(items):
                    lo = lh * 512 + k - 1
                    lhsT = w_t[:, k, cc, :]
                    if lo < 0:
                        rhs = x_t[:, cc, 0:511]
                        outap = ps[:, 1:512]
                    elif lo + 512 > L:
                        rhs = x_t[:, cc, lo:L]
                        outap = ps[:, 0 : L - lo]
                    else:
                        rhs = x_t[:, cc, lo : lo + 512]
                        outap = ps[:, 0:512]
                    nc.tensor.matmul(
                        outap,
                        lhsT,
                        rhs,
                        start=(start and idx == 0),
                        stop=(stop and idx == len(items) - 1),
                    )

            # ---- per head-pair q/k conv units (weights prefetched) ----
            def qk_conv_units(t):
                """Return (q_t, k_t, units): four closures each emitting one
                conv psum-group (~5us of dense PE work) for pair t. Weight
                DMAs are issued immediately (ACT queue), ~20us ahead."""
                q_t = qkpool.tile([128, L], bf16, tag="q", name=f"q{t}")
                k_t = qkpool.tile([128, L], bf16, tag="k", name=f"k{t}")
                wq_t = wpool.tile([128, KW, 8, 128], bf16, tag="w", name="wqt")
                nc.scalar.dma_start(wq_t, wq_d[t])
                wk_t = wpool.tile([128, KW, 8, 128], bf16, tag="w", name="wkt")
                nc.scalar.dma_start(wk_t, wk_d[t])

                def unit(which, lh):
                    ps = convp.tile([128, 512], f32, tag="cp")
                    if which == "q":
                        conv_mms(ps, wq_t, xq_s, range(8), lh, True, True)
                        nc.vector.tensor_scalar_add(
                            q_t[:, lh * 512 : (lh + 1) * 512], ps,
                            qb_s[:, t : t + 1])
                    else:
                        conv_mms(ps, wk_t, xk_s, range(8), lh, True, True)
                        nc.vector.tensor_copy(
                            k_t[:, lh * 512 : (lh + 1) * 512], ps)

                units = [lambda w=w, lh=lh: unit(w, lh)
                         for w in ("q", "k") for lh in range(2)]
                return q_t, k_t, units

            # issue wq0/wk0 DMAs now (behind wv on the ACT queue)
            q_t, k_t, units0 = qk_conv_units(0)

            # ---- V conv, transposed: psum[j, c] = sum_{ci,k} x[ci, j+k-1]
            #      * wv[ci, k, c]; one [128, 512] psum group per j-block ----
            for jb in range(8):
                ps = convp.tile([128, 512], f32, tag="cp")
                J = jb * 128
                items = [(k, cc) for k in (1, 0, 2) for cc in range(8)]
                for idx, (k, cc) in enumerate(items):
                    c0 = J + k  # +1 pad shift, -1 tap offset
                    nc.tensor.matmul(
                        ps,
                        xv_s[:, cc, c0 : c0 + 128],
                        wv_s[:, cc, k, :],
                        start=(idx == 0),
                        stop=(idx == len(items) - 1),
                    )
                nc.vector.tensor_copy(vT[:, jb, 0:8, 0:64], ps[:, 0:512])

            # q/k conv for t=0 (weights already resident)
            for u in units0:
                u()

            # ---- o-conv helpers: partial (cc 0..2) opened during t=3's
            #      exp phase, closed (cc 3) after; rest run normally ----
            o_state = {}

            def oconv_open_units():
                wo_t = wpool.tile([128, KW, 4, 128], bf16, tag="w", name="wo")
                nc.scalar.dma_start(wo_t, wo_d[0])
                o_state["wo"] = wo_t

                def openu(lh):
                    ps = convp.tile([128, 512], f32, tag="cp", name=f"ops{lh}")
                    conv_mms(ps, wo_t, o_x, (0, 1, 2), lh, True, False)
                    o_state[f"ps{lh}"] = ps

                return [lambda lh=lh: openu(lh) for lh in range(2)]

            # ---- per head-pair: software-pipelined q/k conv + attention ----
            # Per pair t: scores+exp(t) -> q/k conv(t+1) [PE work that hides
            # exp(t) on ACT] -> AV+normalize(t).
            for t in range(4):
                if t < 3:
                    nq_t, nk_t, conv_units = qk_conv_units(t + 1)
                else:
                    conv_units = oconv_open_units()
                # scores + exp for both heads; adjacent matmuls of the two
                # heads hit disjoint PE row groups (base 0 / 64) and overlap.
                p_pair = []
                for jc in range(8):
                    sps_pair = [scorep.tile([128, L], f32, tag="score",
                                            name=f"sps{hh2}")
                                for hh2 in range(2)]
                    if jc == 0:
                        p_pair = [ppool.tile([128, 8, L], bf16, tag="p",
                                             name=f"p{hh2}")
                                  for hh2 in range(2)]
                    for ih in range(2):
                        for hh in range(2):
                            base = hh * 64
                            nc.tensor.matmul(
                                sps_pair[hh][:, ih * 512 : (ih + 1) * 512],
                                k_t[base : base + 64, jc * 128 : (jc + 1) * 128],
                                q_t[base : base + 64, ih * 512 : (ih + 1) * 512],
                                start=True,
                                stop=True,
                            )
                    for hh in range(2):
                        nc.scalar.activation(
                            p_pair[hh][:, jc, :], sps_pair[hh], Act.Exp,
                            bias=jb_s[:, jc : jc + 1],
                        )
                    # one conv psum-group (~5us dense PE) after every other
                    # jc-group: fills the PE stall while ACT drains exp(t)
                    if jc % 2 == 1 and conv_units:
                        conv_units.pop(0)()
                # AV + normalize for both heads
                for hh in range(2):
                    h = 2 * t + hh
                    base = hh * 64
                    for ih in range(2):
                        avps = avp.tile([65, 512], f32, tag="av")
                        for jc in range(8):
                            nc.tensor.matmul(
                                avps,
                                vT[:, jc, h, :],
                                p_pair[hh][:, jc, ih * 512 : (ih + 1) * 512],
                                start=(jc == 0),
                                stop=(jc == 7),
                            )
                        r_t = smalls.tile([1, 512], f32, tag="r")
                        nc.vector.reciprocal(r_t, avps[64:65, :])
                        bc_t = smalls.tile([64, 512], f32, tag="bc")
                        nc.gpsimd.partition_broadcast(bc_t, r_t)
                        dst = o_x[base : base + 64, t, ih * 512 : (ih + 1) * 512]
                        if hh == 0:
                            nc.vector.tensor_mul(dst, avps[0:64, :], bc_t)
                        else:
                            tmp = smalls.tile([64, 512], bf16, tag="tmp")
                            nc.vector.tensor_mul(tmp, avps[0:64, :], bc_t)
                            nc.sync.dma_start(dst, tmp)
                if t < 3:
                    q_t, k_t = nq_t, nk_t

            # ---- out conv (partial over this core's 512 input channels) ----
            def o_emit(ps, occ, lh):
                o_t = outp.tile([128, 512], bf16, tag="osb")
                nc.vector.tensor_copy(o_t, ps)
                nc.sync.dma_start(out_d[occ, :, lh * 512 : (lh + 1) * 512], o_t)

            # close the two partially-accumulated occ=0 groups (cc 3)
            wo_next = wpool.tile([128, KW, 4, 128], bf16, tag="w", name="wo")
            nc.scalar.dma_start(wo_next, wo_d[1])
            for lh in range(2):
                ps = o_state[f"ps{lh}"]
                conv_mms(ps, o_state["wo"], o_x, (3,), lh, False, True)
                o_emit(ps, 0, lh)
            for occ in range(1, 8):
                wo_t = wo_next
                if occ < 7:
                    wo_next = wpool.tile([128, KW, 4, 128], bf16, tag="w",
                                         name="wo")
                    nc.scalar.dma_start(wo_next, wo_d[occ + 1])
                for lh in range(2):
                    ps = convp.tile([128, 512], f32, tag="cp")
                    conv_mms(ps, wo_t, o_x, (0, 1, 2, 3), lh, True, True)
                    o_emit(ps, occ, lh)

    nc.compile()
    return nc


def _get_nc():
    if "nc" not in _CACHE:
        _CACHE["nc"] = _build_nc()
    return _CACHE["nc"]


def _prep_inputs(query, key, value, key_padding_mask, attn_mask,
                 q_w, q_b, k_w, k_b, v_w, v_b, o_w, o_b):
    """Build the 8 per-core input maps (host-side shard + layout)."""
    query = np.asarray(query, np.float32)
    key = np.asarray(key, np.float32)
    value = np.asarray(value, np.float32)
    kpm = np.asarray(key_padding_mask)
    attn_mask = np.asarray(attn_mask, np.float32)
    q_w = np.asarray(q_w, np.float32); q_b = np.asarray(q_b, np.float32)
    k_w = np.asarray(k_w, np.float32)
    v_w = np.asarray(v_w, np.float32)
    o_w = np.asarray(o_w, np.float32); o_b = np.asarray(o_b, np.float32)

    # attn_mask must be constant across query rows to fold into the key bias
    if not np.all(attn_mask == attn_mask[0:1, :]):
        raise NotImplementedError("attn_mask varying over query index unsupported")
    am_row = attn_mask[0]

    def conv_w_layout(w, occ, n_ci):
        # w: [C_out_part, C_in_part, KW] -> [occ, p(ci), k, cc, m(c_out)]
        co, ci, _ = w.shape
        arr = w.reshape(occ, 128, n_ci, 128, KW).transpose(0, 3, 4, 2, 1)
        return np.ascontiguousarray(arr).astype(BF16)

    wq_h, wk_h, wv_h, wo_h, qb_h = [], [], [], [], []
    for hg in range(2):
        sl = slice(hg * HALF, (hg + 1) * HALF)
        wq_h.append(conv_w_layout(q_w[sl] * SCALE, 4, 8))
        wk_h.append(conv_w_layout(k_w[sl], 4, 8))
        # transposed-v layout: [cc, ci, k, c_local]
        wv_h.append(np.ascontiguousarray(
            v_w[sl].transpose(1, 2, 0).reshape(8, 128, KW, 512)).astype(BF16))
        # out conv: contract over this half's input channels
        wo_h.append(conv_w_layout(o_w[:, sl, :], 8, 4))
        qb_h.append(np.ascontiguousarray(
            (q_b[sl] * SCALE).reshape(4, 128).T).astype(np.float32))

    xq_b, xk_b, xv_b, jb_b = [], [], [], []
    for b in range(B):
        xq_b.append(np.ascontiguousarray(query[b].T).reshape(8, 128, L).astype(BF16))
        xk_b.append(np.ascontiguousarray(key[b].T).reshape(8, 128, L).astype(BF16))
        xv_b.append(np.ascontiguousarray(value[b].T).reshape(8, 128, L).astype(BF16))
        jb = np.where(kpm[b], MASK_BIAS, 0.0).astype(np.float32) + am_row
        jb_b.append(np.ascontiguousarray(jb.reshape(8, 128).T).astype(np.float32))

    in_maps = []
    for c in range(NCORES):
        b, hg = c // 2, c % 2
        in_maps.append({
            "xq": xq_b[b], "xk": xk_b[b], "xv": xv_b[b],
            "wq": wq_h[hg], "wk": wk_h[hg], "wv": wv_h[hg], "wo": wo_h[hg],
            "qb": qb_h[hg], "jb": jb_b[b],
        })
    return in_maps, (o_w, np.asarray(v_b, np.float32), o_b)


def _postprocess(parts, extras):
    """parts: list of 8 arrays [8,128,L] bf16 -> full output [B, L, D] f32."""
    o_w, v_b, o_b = extras
    # v-bias contribution through the out conv (attention rows sum to 1):
    # interior columns see all 3 taps, edge columns lose one.
    a_full = o_w.sum(axis=2) @ v_b            # [D]
    a_l0 = a_full - o_w[:, :, 0] @ v_b        # l = 0 loses tap k=0
    a_lL = a_full - o_w[:, :, 2] @ v_b        # l = L-1 loses tap k=2
    out = np.empty((B, L, D), np.float32)
    for b in range(B):
        tot = (parts[2 * b].astype(np.float32)
               + parts[2 * b + 1].astype(np.float32)).reshape(D, L)
        tot = tot + o_b[:, None] + a_full[:, None]
        tot[:, 0] += a_l0 - a_full
        tot[:, -1] += a_lL - a_full
        out[b] = tot.T
    return out


def _run(in_maps, trace=False, **kw):
    from concourse import bass_utils
    nc = _get_nc()
    try:
        res = bass_utils.run_bass_kernel_spmd(
            nc, in_maps, core_ids=list(range(NCORES)), trace=trace, **kw)
    except ModuleNotFoundError:
        # NTFF profiling hook unavailable (axon client without axon.trn);
        # rerun without trace.
        res = bass_utils.run_bass_kernel_spmd(
            nc, in_maps, core_ids=list(range(NCORES)), trace=False, **kw)
    return res


def kernel(**inputs) -> np.ndarray:
    in_maps, extras = _prep_inputs(**inputs)
    res = _run(in_maps, trace=bool(int(os.environ.get("KERNEL_TRACE", "0"))))
    parts = [res.results[c]["out"] for c in range(NCORES)]
    out = _postprocess(parts, extras)
    if res.exec_time_ns is not None:
        print(f"HW exec time: {res.exec_time_ns} ns")
    return out
